# revision 44
# baseline (speedup 1.0000x reference)
"""Trainium2 Bass kernel for nn_MAB_2121713844542 (dense transformer block).

Data-parallel over batch B=32 across 8 cores (4 batches/core), activations
transposed [feature, seq] so every matmul contracts on partitions.

v2 changes over the 280us baseline:
  - FFN1/FFN2 in fp8e4 DoubleRow: one matmul contracts BOTH 128-row k-tiles
    (weights hold 2 rows/PE cell), halving FFN PE instructions.  W1/W2
    pre-scaled x16 into e4m3 normal range; the 1/16 folds into the gelu's
    input scale and the Z assembly's scalar_tensor_tensor.
  - fp16 instead of bf16 for all activation/weight plumbing (same 2B/cycle
    PE stream rate, 8x finer mantissa) EXCEPT expS (exp up to e^11 would
    overflow fp16) and its matmul partners Ed4/Vh which stay bf16.
  - LayerNorm stats/broadcast matmuls moved off f32r moving data (which
    streams at half rate): x tiles (OT/Z) are f16, x^2 comes from GpSimd in
    f16, rstd/cst/one33/g rows are f16 -> every LN matmul streams at full
    rate.
  - Input DMA for batch b+1 is enqueued BEFORE batch b-1's output DMA (the
    in-order sync queue previously parked the load behind the store, so the
    proj filler meant to cover LN1's stats chain had no data).
  - Attention split by quad: quad 0 of batch b+1 interleaves with batch b's
    gelu window (the G loop is ACT-paced at ~570ns/tile vs 256ns of PE work,
    so the PE otherwise idles there); quad 1 stays in LN2(b)'s filler.
  - Activation-table thrash fix kept from v1 (only natural_log_exp + gelu
    tables, dummy activations prefetch the switches).
"""

import functools

import numpy as np
import ml_dtypes

import concourse.bass as bass
import concourse.mybir as mybir
import concourse.tile as tile
from concourse import bacc
from concourse import hw_specs as _hw_specs
from concourse.bass_utils import run_bass_kernel_spmd

_KEEP_TABLES = ("natural_log_exp_and_others", "gelu_and_others")
_orig_get_tables = _hw_specs.get_activation_tables


@functools.cache
def _patched_get_tables(arch):
    tabs = _orig_get_tables(arch)
    return {k: (v if k in _KEEP_TABLES else set()) for k, v in tabs.items()}


_hw_specs.get_activation_tables = _patched_get_tables
bacc.get_activation_tables = _patched_get_tables

B, S, D, H, DH, DFF = 32, 512, 256, 8, 32, 2048
NCORES = 8
BL = B // NCORES
P = 128
DT = D // P     # 2 feature tiles
FT = DFF // P   # 16 ffn tiles
ST = S // P     # 4 seq tiles
f32 = mybir.dt.float32
f32r = mybir.dt.float32r
bf16 = mybir.dt.bfloat16
f16 = mybir.dt.float16
f8 = mybir.dt.float8e4
AF = mybir.ActivationFunctionType
ALU = mybir.AluOpType
DR = mybir.MatmulPerfMode.DoubleRow
EPS = 1e-5
W8SC = 16.0  # fp8 weight pre-scale


def build_nc(beta_zero):
    nc = bacc.Bacc("TRN2", target_bir_lowering=False, debug=False,
                   num_devices=NCORES)

    QT = nc.dram_tensor("QT", (BL, P, DT, S), f16, kind="ExternalInput")
    KT = nc.dram_tensor("KT", (BL, P, DT, S), f16, kind="ExternalInput")
    pT = nc.dram_tensor("pT", (BL, 4, S), f16, kind="ExternalInput")
    Wq = nc.dram_tensor("Wq", (P, DT, D), f16, kind="ExternalInput")
    Wv = nc.dram_tensor("Wv", (P, DT, D), f16, kind="ExternalInput")
    WqA = nc.dram_tensor("WqA", (P, DT, 4, P), f16, kind="ExternalInput")
    WkA = nc.dram_tensor("WkA", (P, DT, 4, P), f16, kind="ExternalInput")
    WpAq = nc.dram_tensor("WpAq", (4, 4, P), f16, kind="ExternalInput")
    WpAk = nc.dram_tensor("WpAk", (4, 4, P), f16, kind="ExternalInput")
    W1 = nc.dram_tensor("W1", (P, DT, DFF), f8, kind="ExternalInput")
    W2b = nc.dram_tensor("W2b", (P, FT, D), f8, kind="ExternalInput")
    bq = nc.dram_tensor("bq", (P, DT), f32, kind="ExternalInput")
    bvb = nc.dram_tensor("bvb", (P, D), f32, kind="ExternalInput")
    b1 = nc.dram_tensor("b1", (P, FT), f32, kind="ExternalInput")
    b2 = nc.dram_tensor("b2", (P, DT), f32, kind="ExternalInput")
    g0r = nc.dram_tensor("g0r", (1, D), f16, kind="ExternalInput")
    nb0 = nc.dram_tensor("nb0", (1, D), f16, kind="ExternalInput")
    g1r = nc.dram_tensor("g1r", (1, D), f16, kind="ExternalInput")
    nb1 = nc.dram_tensor("nb1", (1, D), f16, kind="ExternalInput")
    one33 = nc.dram_tensor("one33", (P, 2, 33), f16, kind="ExternalInput")
    Ed4 = nc.dram_tensor("Ed4", (P, 4, 4), bf16, kind="ExternalInput")
    EB4 = nc.dram_tensor("EB4", (4, P), bf16, kind="ExternalInput")
    onesS = nc.dram_tensor("onesS", (1, S), f16, kind="ExternalInput")
    outT = nc.dram_tensor("outT", (BL, P, DT, S), f16, kind="ExternalOutput")

    with tile.TileContext(nc) as tc:
        with (
            tc.tile_pool(name="singles", bufs=1) as singles,
            tc.tile_pool(name="dbl", bufs=2) as dbl,
            tc.tile_pool(name="ps_mm", bufs=2, space="PSUM") as ps_mm,
            tc.tile_pool(name="ps_sc", bufs=2, space="PSUM") as ps_sc,
            tc.tile_pool(name="ps_acc", bufs=1, space="PSUM") as ps_acc,
            tc.tile_pool(name="ps_av", bufs=1, space="PSUM") as ps_av,
        ):
            def load(dram, shape, eng=None):
                t = singles.tile(list(shape), dram.dtype, name="w_" + dram.name)
                (eng or nc.sync).dma_start(
                    t, dram[tuple(slice(None) for _ in shape)])
                return t

            # order matters: only what batch 0's proj needs loads first;
            # the FFN weights stream in during batch-0 attention
            Wq_sb = load(Wq, (P, DT, D))

            def loadj(dram, shape):
                st = load(dram, shape)
                t = singles.tile(list(shape), f32, name="j_" + dram.name)
                nc.vector.tensor_copy(t, st)
                return t

            bq_sb = loadj(bq, (P, DT))

            eps1 = singles.tile([1, 1], f32)
            nc.vector.memset(eps1, EPS)
            neghalf = singles.tile([1, 1], f32)
            nc.vector.memset(neghalf, -0.5)
            dummy = singles.tile([1, 1], f32)
            nc.vector.memset(dummy, 1.0)

            def ln_stats(x_sb):
                """Per-token mean/var stats chain -> (rstd, cst) tiles."""
                x2 = dbl.tile([P, DT, S], f16, tag="x2", bufs=1, name="x2")
                for t in range(DT):
                    nc.gpsimd.tensor_tensor(x2[:, t, :], x_sb[:, t, :],
                                            x_sb[:, t, :], ALU.mult)
                # partition 0 <- mean, partition 32 <- E[x^2]
                acc = ps_acc.tile([33, S], f32, tag="acc", name="acc")
                for t in range(DT):
                    nc.tensor.matmul(acc, one33_sb[:, 0, :], x_sb[:, t, :],
                                     start=(t == 0), stop=False)
                for t in range(DT):
                    nc.tensor.matmul(acc, one33_sb[:, 1, :], x2[:, t, :],
                                     start=False, stop=(t == DT - 1))
                rstd = dbl.tile([1, S], f16, tag="rstd", name="rstd")
                m2v = dbl.tile([1, S], f32r, tag="m2v", name="m2v")
                cst = dbl.tile([1, S], f16, tag="cst", name="cst")
                nc.scalar.activation(m2v, acc[0:1, :], AF.Square)
                nc.vector.tensor_sub(m2v, acc[32:33, :], m2v)
                nc.scalar.activation(acc[32:33, :], m2v, AF.Ln, bias=eps1)
                # rstd = exp(-0.5*ln(var+eps))
                nc.scalar.activation(rstd, acc[32:33, :], AF.Exp,
                                     scale=neghalf)
                # C = mean * rstd
                nc.vector.tensor_mul(cst, acc[0:1, :], rstd)
                return rstd, cst

            def layer_norm(x_sb, grow, nbrow, out_sb, filler=None,
                           split=False, sink=None, pre=None):
                """out = LN(x) * g + beta.  x_sb [P,DT,S] f16."""
                rstd, cst = pre if pre is not None else ln_stats(x_sb)
                layer_norm.rstd = rstd
                # independent matmuls emitted here keep the PE fed while the
                # Square->sub->Ln->Exp stats chain resolves (in-order queue)
                if filler is not None:
                    filler()
                # split=True emits the broadcast+tail per half-token chunk so
                # the final output DMAs start earlier (last-batch drain)
                halves = ((slice(0, S // 2), slice(S // 2, S))
                          if split else (slice(0, S),))
                for t in range(DT):
                    for sl in halves:
                        bcAC = ps_sc.tile([P, 2, S], f32, tag="sc",
                                          name="bcAC")
                        bcA, bcC = bcAC[:, 0, sl], bcAC[:, 1, sl]
                        nc.tensor.matmul(bcA, grow[0:1, t * P:(t + 1) * P],
                                         rstd[:, sl], start=True, stop=True)
                        nc.tensor.matmul(bcC, grow[0:1, t * P:(t + 1) * P],
                                         cst[:, sl], start=True,
                                         stop=beta_zero)
                        if not beta_zero:
                            nc.tensor.matmul(
                                bcC, nbrow[0:1, t * P:(t + 1) * P],
                                onesS_sb[:, sl], start=False, stop=True)
                        # out = x*(g*rstd) - (g*mean*rstd - beta)
                        nc.vector.tensor_mul(out_sb[:, t, sl],
                                             x_sb[:, t, sl], bcA)
                        nc.vector.tensor_sub(out_sb[:, t, sl],
                                             out_sb[:, t, sl], bcC)
                        if sink is not None:
                            sink(t, sl)

            def stage_load(b, stt):
                QT_sb = dbl.tile([P, DT, S], f16, tag="qt", name="QT_sb")
                nc.sync.dma_start(QT_sb, QT[b])
                KT_sb = dbl.tile([P, DT, S], f16, tag="kt", name="KT_sb")
                nc.sync.dma_start(KT_sb, KT[b])
                pT_sb = dbl.tile([4, S], f16, tag="pt", name="pT_sb")
                nc.sync.dma_start(pT_sb, pT[b])
                stt.update(QT=QT_sb, KT=KT_sb, pT=pT_sb)

            def stage_proj(b, stt):
                QT_sb, KT_sb, pT_sb = stt["QT"], stt["KT"], stt["pT"]
                # natural Qh (for the attention residual)
                Qh = dbl.tile([P, DT, S], f16, tag="qh", name="Qh")
                for t in range(DT):
                    ps = ps_mm.tile([P, S], f32, tag="mm", name="psq")
                    for kt in range(DT):
                        nc.tensor.matmul(
                            ps, Wq_sb[:, kt, t * P:(t + 1) * P],
                            QT_sb[:, kt, :],
                            start=(kt == 0), stop=(kt == DT - 1))
                    nc.vector.tensor_tensor(
                        Qh[:, t, :], ps,
                        bq_sb[:, t:t + 1].to_broadcast((P, S)), ALU.add)
                # aug tiles for scores: tile j partitions =
                # [Qh(2j) | Ph(2j) | Qh(2j+1) | Ph(2j+1)], biases folded via
                # the pT ones-row, so one K=64 matmul per (head, kt) yields
                # QK^T + PP^T in a single accumulation
                QA = dbl.tile([P, 4, S], f16, tag="qa", name="QA")
                KA = dbl.tile([P, 4, S], f16, tag="ka", name="KA")
                for j in range(4):
                    ps = ps_mm.tile([P, S], f32, tag="mm", name="psqa")
                    for kt in range(DT):
                        nc.tensor.matmul(ps, WqA_sb[:, kt, j, :],
                                         QT_sb[:, kt, :],
                                         start=(kt == 0), stop=False)
                    nc.tensor.matmul(ps, WpAq_sb[:, j, :], pT_sb,
                                     start=False, stop=True)
                    nc.vector.tensor_copy(QA[:, j, :], ps)
                    ps = ps_mm.tile([P, S], f32, tag="mm", name="pska")
                    for kt in range(DT):
                        nc.tensor.matmul(ps, WkA_sb[:, kt, j, :],
                                         KT_sb[:, kt, :],
                                         start=(kt == 0), stop=False)
                    nc.tensor.matmul(ps, WpAk_sb[:, j, :], pT_sb,
                                     start=False, stop=True)
                    nc.vector.tensor_copy(KA[:, j, :], ps)

                # V in natural layout [keys, feat], bf16, bias fused in move
                Vh = dbl.tile([P, ST, D], bf16, tag="vh", name="Vh")
                for st in range(ST):
                    ps = ps_mm.tile([P, S], f32, tag="mm", name="psv")
                    for kt in range(DT):
                        nc.tensor.matmul(
                            ps[:, :D], KT_sb[:, kt, st * P:(st + 1) * P],
                            Wv_sb[:, kt, :],
                            start=(kt == 0), stop=(kt == DT - 1))
                    nc.vector.tensor_add(Vh[:, st, :], ps[:, :D], bvb_sb)
                stt.update(Qh=Qh, QA=QA, KA=KA, Vh=Vh)

            def attn_quad_scores(b, stt, quad, ktp):
                """Scores + exp for one (quad, key-tile-pair)."""
                QA, KA = stt["QA"], stt["KA"]
                expS = stt["expS"][quad]
                for h4 in range(4):
                    base = 64 * (h4 % 2)
                    j = 2 * quad + h4 // 2
                    ps2 = ps_sc.tile([P, 2, S], f32, tag="sc", name="pssc")
                    for k2 in range(2):
                        kt = 2 * ktp + k2
                        nc.tensor.matmul(
                            ps2[:, k2, :],
                            KA[base:base + 64, j, kt * P:(kt + 1) * P],
                            QA[base:base + 64, j, :],
                            start=True, stop=True,
                            tile_position=(base, 0))
                    nc.scalar.activation(
                        expS[h4][:, 2 * ktp:2 * ktp + 2, :],
                        ps2, AF.Exp)

            def attn_quad_denav(b, stt, quad, ktp):
                Vh = stt["Vh"]
                expS = stt["expS"][quad]
                den, av = stt["den"][quad], stt["av"][quad]
                for h4 in range(4):
                    h = 4 * quad + h4
                    for k2 in range(2):
                        kt = 2 * ktp + k2
                        nc.tensor.matmul(
                            den, Ed4_sb[:, h4, :],
                            expS[h4][:, kt, :],
                            start=(kt == 0 and h4 == 0),
                            stop=(kt == ST - 1 and h4 == 3),
                            skip_group_check=True)
                        nc.tensor.matmul(
                            av[32 * h4:32 * h4 + 32, :],
                            Vh[:, kt, 32 * h:32 * h + 32],
                            expS[h4][:, kt, :],
                            start=(kt == 0), stop=(kt == ST - 1),
                            tile_position=(0, 32 * h4),
                            skip_group_check=True)

            def attn_quad_out(b, stt, quad):
                """1/den broadcast, divide, +Qh residual -> OT[:, quad, :]."""
                Qh, OT = stt["Qh"], stt["OT"]
                den, av = stt["den"][quad], stt["av"][quad]
                r4f = dbl.tile([4, S], f32, tag="r4f", name="r4f")
                nc.vector.reciprocal_approx_fast(r4f, den[0:4, :])
                r4 = dbl.tile([4, S], bf16, tag="r4", name="r4")
                nc.vector.tensor_copy(r4, r4f)
                bc2 = ps_sc.tile([P, 2, S], f32, tag="sc", name="bc2")
                bc = bc2[:, 0, :]
                nc.tensor.matmul(bc, EB4_sb, r4, start=True, stop=True)
                bcS = dbl.tile([P, S], f32, tag="bcs", name="bcS")
                nc.vector.tensor_copy(bcS, bc)
                nc.vector.tensor_mul(OT[:, quad, :], av, bcS)
                nc.vector.tensor_add(OT[:, quad, :], OT[:, quad, :],
                                     Qh[:, quad, :])

            def attn_alloc(stt):
                stt["OT"] = dbl.tile([P, DT, S], f16, tag="ot", name="OT")
                stt["expS"] = {}
                stt["den"] = {}
                stt["av"] = {}

            def attn_quad_alloc(stt, quad):
                stt["expS"][quad] = [
                    dbl.tile([P, ST, S], bf16, tag=f"e{i}", name=f"expS{i}")
                    for i in range(4)]
                stt["den"][quad] = ps_acc.tile([4, S], f32, tag="acc",
                                               name="den")
                stt["av"][quad] = ps_av.tile([P, S], f32, tag="av", name="av")

            def attn_scores_exps(b, stt, quad):
                """Scores + exps for a quad — MUST run in an ln_exp-table
                window (LN1/LN2 stats fillers), never inside the gelu loop,
                or the act-table chooser inserts 1.3us switches."""
                attn_quad_alloc(stt, quad)
                for ktp in range(ST // 2):
                    attn_quad_scores(b, stt, quad, ktp)

            def attn_denav(b, stt, quad):
                """den/av matmuls — PE-only (no ACT), safe anywhere."""
                for ktp in range(ST // 2):
                    attn_quad_denav(b, stt, quad, ktp)

            def stage_ffn(b, stt, nxt):
                OT = stt["OT"]
                # finish quad 1 of this batch's attention (scores+exps ran
                # in LN2(b-1)'s window) so OT is complete before LN1 stats
                attn_denav(b, stt, 1)
                attn_quad_out(b, stt, 1)

                LN1 = dbl.tile([P, DT, S], f16, tag="ln1", name="LN1")

                def filler():
                    if nxt is not None:
                        if not nxt.pop("proj_done", False):
                            stage_proj(b + 1, nxt)
                        attn_alloc(nxt)
                        attn_scores_exps(b + 1, nxt, 0)
                layer_norm(OT, g0_sb, nb0_sb, LN1, filler=filler)
                # fp8 copy of LN1 for the DoubleRow FFN1 matmuls; per-t so
                # the t=0 copy overlaps the t=1 LN tail on the DVE queue
                LN1q = dbl.tile([P, DT, S], f8, tag="ln1q", name="LN1q")
                for t in range(DT):
                    nc.vector.tensor_copy(LN1q[:, t, :], LN1[:, t, :])
                # prefetch the gelu table; input dep on LN1's rstd pins this
                # after LN1's Exp in the ACT queue (scheduler can't hoist it)
                nc.scalar.activation(dummy, layer_norm.rstd[0:1, 0:1],
                                     AF.Gelu)

                # G loop (ACT-paced ~570ns/gelu vs 256ns of PE work per ft):
                # interleave batch b+1's den/av (PE-only) to keep the PE fed
                G = dbl.tile([P, FT, S], f8, tag="g", bufs=1, name="G")

                def g_chunk(ft0, ft1):
                    for ft in range(ft0, ft1):
                        ps = ps_mm.tile([P, S], f32, tag="mm", name="psf")
                        nc.tensor.matmul(
                            ps, W1_sb[:, :, ft * P:(ft + 1) * P],
                            LN1q, start=True, stop=True, perf_mode=DR)
                        nc.scalar.activation(G[:, ft, :], ps, AF.Gelu,
                                             bias=b1_sb[:, ft:ft + 1],
                                             scale=1.0 / W8SC)

                if nxt is not None:
                    # denav ktp0 mixes into the ACT-paced gelu window; ktp1
                    # mixes into the FFN2 stretch (spreads the hot fp8
                    # streams between power-light M=32/M=4 matmuls so the
                    # duty-cycle governor stays at full rate)
                    g_chunk(0, 8)
                    attn_quad_denav(b + 1, nxt, 0, 0)
                    g_chunk(8, FT)
                else:
                    g_chunk(0, FT)
                # prefetch the ln/exp table; dep on the last gelu's output
                # pins it after the gelu loop in the ACT queue
                nc.scalar.activation(dummy, G[0:1, FT - 1, 0:1], AF.Ln)
                Z = dbl.tile([P, DT, S], f16, tag="z", bufs=1, name="Z")
                for t in range(DT):
                    ps = ps_mm.tile([P, S], f32, tag="mm", name="psf2")
                    for fp in range(FT // 2):
                        nc.tensor.matmul(
                            ps, W2_sb[:, 2 * fp:2 * fp + 2,
                                      t * P:(t + 1) * P],
                            G[:, 2 * fp:2 * fp + 2, :],
                            start=(fp == 0), stop=(fp == FT // 2 - 1),
                            perf_mode=DR)
                    if t == 0 and nxt is not None:
                        attn_quad_denav(b + 1, nxt, 0, 1)
                    # Z = ps/16 + LN1 + b2
                    nc.vector.scalar_tensor_tensor(
                        Z[:, t, :], ps, 1.0 / W8SC, LN1[:, t, :],
                        ALU.mult, ALU.add)
                    nc.vector.tensor_tensor(
                        Z[:, t, :], Z[:, t, :],
                        b2_sb[:, t:t + 1].to_broadcast((P, S)), ALU.add)
                if nxt is not None:
                    attn_quad_out(b + 1, nxt, 0)
                stt["Z"] = Z

            def stage_out(b, stt, nxt):
                OUT = dbl.tile([P, DT, S], f16, tag="out", name="OUT")
                filler = None
                if nxt is not None:
                    def filler():
                        # input DMA for b+2 enqueued before this batch's
                        # output DMA (in-order sync queue head-of-line fix)
                        if b + 2 < BL:
                            stage_load(b + 2, sts[b + 2])
                        attn_scores_exps(b + 1, nxt, 1)
                    layer_norm(stt["Z"], g1_sb, nb1_sb, OUT, filler=filler)
                    for t in range(DT):
                        nc.sync.dma_start(outT[b][:, t, :], OUT[:, t, :])
                else:
                    # last batch: half-token tail so the output DMAs start
                    # as soon as each chunk's normalize completes
                    def sink(t, sl):
                        nc.sync.dma_start(outT[b][:, t, sl], OUT[:, t, sl])
                    layer_norm(stt["Z"], g1_sb, nb1_sb, OUT,
                               split=True, sink=sink)

            # software pipeline (steady state, per batch b):
            #   [denav+out q1(b)] [LN1 stats | proj(b+1) + scores/exps
            #   q0(b+1)] [LN1 bc] [gelu window # denav+out q0(b+1)]
            #   [FFN2] [LN2 stats | load(b+2) + scores/exps q1(b+1)]
            #   [LN2 bc] [outT]
            # so exps only ever run inside ln_exp-table windows and the
            # ACT-paced gelu stream overlaps PE-only den/av work.
            sts = [dict() for _ in range(BL)]
            stage_load(0, sts[0])
            # big weights go on the gpsimd SWDGE queue so they don't
            # serialize with the input/output DMAs on the sync queue
            WqA_sb = load(WqA, (P, DT, 4, P), eng=nc.gpsimd)
            WkA_sb = load(WkA, (P, DT, 4, P), eng=nc.gpsimd)
            WpAq_sb = load(WpAq, (4, 4, P))
            WpAk_sb = load(WpAk, (4, 4, P))
            Wv_sb = load(Wv, (P, DT, D), eng=nc.gpsimd)
            bvb_sb = loadj(bvb, (P, D))
            Ed4_sb = load(Ed4, (P, 4, 4))
            EB4_sb = load(EB4, (4, P))
            one33_sb = load(one33, (P, 2, 33))
            onesS_sb = load(onesS, (1, S))
            g0_sb = load(g0r, (1, D))
            nb0_sb = load(nb0, (1, D))
            g1_sb = load(g1r, (1, D))
            nb1_sb = load(nb1, (1, D))
            if BL > 1:
                stage_load(1, sts[1])
            stage_proj(0, sts[0])
            attn_alloc(sts[0])
            attn_scores_exps(0, sts[0], 0)
            attn_denav(0, sts[0], 0)
            attn_quad_out(0, sts[0], 0)
            attn_scores_exps(0, sts[0], 1)
            if BL > 1:
                # proj(1) here fills the PE while batch 0's quad-1 exps
                # stream on ACT (the last uncovered startup window)
                stage_proj(1, sts[1])
                sts[1]["proj_done"] = True
            W1_sb = load(W1, (P, DT, DFF), eng=nc.gpsimd)
            W2_sb = load(W2b, (P, FT, D), eng=nc.gpsimd)
            b1_sb = loadj(b1, (P, FT))
            b2_sb = loadj(b2, (P, DT))
            for b in range(BL):
                nxt = sts[b + 1] if b + 1 < BL else None
                stage_ffn(b, sts[b], nxt)
                stage_out(b, sts[b], nxt)

    nc.finalize()
    return nc


_NC = None


def kernel(Q, K, p, Wq, bq, Wk, bk, Wv, bv, Wp, bp, g0, beta0, W1, b1, W2, b2,
           g1, beta1):
    global _NC
    beta_zero = bool(np.all(np.asarray(beta0) == 0)
                     and np.all(np.asarray(beta1) == 0))
    if _NC is None:
        _NC = build_nc(beta_zero)

    f = np.float32
    h16 = np.float16
    bf = ml_dtypes.bfloat16
    q8 = ml_dtypes.float8_e4m3

    def to8(x):
        return np.clip(np.asarray(x, f), -240.0, 240.0).astype(q8)

    def feat_tiles(x):  # [B, S, D] -> [B, P, DT, S]
        x = np.asarray(x, f).transpose(0, 2, 1).reshape(-1, DT, P, S)
        return np.ascontiguousarray(x.transpose(0, 2, 1, 3))

    def pp(vec, n):  # [n*P] -> [P, n]
        return np.ascontiguousarray(np.asarray(vec, f).reshape(n, P).T)

    def wmat(w, n, m):  # [n*P, m] -> [P, n, m]
        w = np.asarray(w, f).reshape(n, P, m)
        return np.ascontiguousarray(w.transpose(1, 0, 2))

    QTf = feat_tiles(Q)
    KTf = feat_tiles(K)
    # p padded to 4 channels; row 3 = ones (carries the PE-proj bias).
    # PE projection pre-scaled by 1/4 so PhPh^T carries the 1/sqrt(DV)=1/16.
    pTf = np.zeros((B, 4, S), f)
    pTf[:, :3, :] = np.transpose(np.asarray(p, f), (0, 2, 1))
    pTf[:, 3, :] = 1.0
    Wq_f = np.asarray(Wq, f)
    Wk_f = np.asarray(Wk, f)
    Wp_f = np.asarray(Wp, f)
    bq_f = np.asarray(bq, f)
    bk_f = np.asarray(bk, f)
    bp_f = np.asarray(bp, f)

    def aug_w(W):  # [D, D] -> [P, DT, 4, P] lhsT tiles
        out = np.zeros((P, DT, 4, P), f)
        Wt = W.reshape(DT, P, D)  # [kt, row, out_feature]
        for j in range(4):
            for hh in range(2):
                h = 2 * j + hh
                out[:, :, j, 64 * hh:64 * hh + 32] = \
                    Wt[:, :, 32 * h:32 * h + 32].transpose(1, 0, 2)
        return out

    def aug_p(bias, qside):
        """[4, 4, P] pT-projection lhsT.  The PE term (Ph+bp)(Ph+bp)^T/16 is
        rank 4 in p4=[p|1], so the aug slots carry 4 values instead of 32:
        q side gets u = p4 @ M4 (M4 = Wp4 Wp4^T / 16), k side gets p4
        verbatim; slots 36-63 stay zero (zero rows cost no PE power).
        Row 3 (pT's ones row) also broadcasts bq/bk into the Qh/Kh slots."""
        out = np.zeros((4, 4, P), f)
        for j in range(4):
            for hh in range(2):
                h = 2 * j + hh
                Wp4 = np.concatenate(
                    [Wp_f[:, 32 * h:32 * h + 32],
                     bp_f[None, 32 * h:32 * h + 32]], axis=0)  # [4, 32]
                if qside:
                    out[:, j, 64 * hh + 32:64 * hh + 36] = \
                        (Wp4 @ Wp4.T) / 16.0
                else:
                    for c in range(4):
                        out[c, j, 64 * hh + 32 + c] = 1.0
                out[3, j, 64 * hh:64 * hh + 32] += bias[32 * h:32 * h + 32]
        return out

    # EB4: r4 row h4 -> out partitions 32*h4..32*h4+31
    EB4m = np.zeros((4, P), f)
    for h4 in range(4):
        EB4m[h4, 32 * h4:32 * h4 + 32] = 1.0
    # Ed4[:, h4, :]: all-ones col h4 (masked partition-sum lhsT)
    Ed4m = np.zeros((P, 4, 4), f)
    for h4 in range(4):
        Ed4m[:, h4, h4] = 1.0
    # LN partition-sum weights (1/D folded in): [:,0,:] puts sum(x)/D at
    # out partition 0, [:,1,:] puts sum(x^2)/D at out partition 32
    one33m = np.zeros((P, 2, 33), f)
    one33m[:, 0, 0] = 1.0 / D
    one33m[:, 1, 32] = 1.0 / D

    shared = {
        "Wq": wmat(Wq, DT, D).astype(h16),
        "Wv": wmat(Wv, DT, D).astype(h16),
        "WqA": aug_w(Wq_f).astype(h16), "WkA": aug_w(Wk_f).astype(h16),
        "WpAq": aug_p(bq_f, True).astype(h16),
        "WpAk": aug_p(bk_f, False).astype(h16),
        "W1": to8(wmat(W1, DT, DFF) * W8SC),
        "W2b": to8(wmat(W2, FT, D) * W8SC),
        "bq": pp(bq, DT),
        "bvb": np.ascontiguousarray(np.broadcast_to(np.asarray(bv, f), (P, D))),
        "b1": pp(b1, FT), "b2": pp(b2, DT),
        "g0r": np.asarray(g0, f).reshape(1, D).astype(h16),
        "nb0": -np.asarray(beta0, f).reshape(1, D).astype(h16),
        "g1r": np.asarray(g1, f).reshape(1, D).astype(h16),
        "nb1": -np.asarray(beta1, f).reshape(1, D).astype(h16),
        "one33": one33m.astype(h16),
        "Ed4": Ed4m.astype(bf), "EB4": EB4m.astype(bf),
        "onesS": np.ones((1, S), h16),
    }
    in_maps = []
    for c in range(NCORES):
        m = dict(shared)
        m["QT"] = np.ascontiguousarray(QTf[c * BL:(c + 1) * BL]).astype(h16)
        m["KT"] = np.ascontiguousarray(KTf[c * BL:(c + 1) * BL]).astype(h16)
        m["pT"] = np.ascontiguousarray(pTf[c * BL:(c + 1) * BL]).astype(h16)
        in_maps.append(m)

    import os
    trace = bool(os.environ.get("BASS_TRACE"))
    res = run_bass_kernel_spmd(_NC, in_maps, core_ids=list(range(NCORES)),
                               trace=trace)
    kernel._LAST = res
    outs = [np.asarray(res.results[c]["outT"], np.float32)
            for c in range(NCORES)]
    full = np.concatenate(outs, axis=0)  # [B, P, DT, S]
    full = full.transpose(0, 2, 1, 3).reshape(B, D, S)  # [B, D, S]
    return np.ascontiguousarray(full.transpose(0, 2, 1))


# revision 46
# speedup vs baseline: 1.0431x; 1.0431x over previous
"""Trainium2 Bass kernel for nn_MAB_2121713844542 (dense transformer block).

Data-parallel over batch B=32 across 8 cores (4 batches/core), activations
transposed [feature, seq] so every matmul contracts on partitions.

Final configuration (~245-248us vs 280us reported / 333us same-device
baseline; rel err 8.9e-3 vs the 2e-2 gate):
  - FFN1/FFN2 in fp8e4 DoubleRow: one matmul contracts BOTH 128-row k-tiles
    (weights hold 2 rows/PE cell), halving FFN PE instructions.  W1/W2
    pre-scaled x16 into e4m3 normal range; the 1/16 folds into the gelu's
    input scale and the Z assembly's scalar_tensor_tensor.
  - fp16 instead of bf16 for all activation/weight plumbing (same 2B/cycle
    PE stream rate, 8x finer mantissa) EXCEPT expS (exp up to e^11 would
    overflow fp16) and its matmul partners Ed4/Vh which stay bf16.
  - LayerNorm stats/broadcast matmuls moved off f32r moving data (which
    streams at half rate): x tiles (OT/Z) are f16, x^2 comes from GpSimd in
    f16, rstd/cst/one33/g rows are f16 -> every LN matmul streams at full
    rate.
  - Input DMA for batch b+1 is enqueued BEFORE batch b-1's output DMA (the
    in-order sync queue previously parked the load behind the store, so the
    proj filler meant to cover LN1's stats chain had no data).
  - Attention split by quad: quad 0 of batch b+1 interleaves with batch b's
    gelu window (the G loop is ACT-paced at ~570ns/tile vs 256ns of PE work,
    so the PE otherwise idles there); quad 1 stays in LN2(b)'s filler.
  - Activation-table thrash fix kept from v1 (only natural_log_exp + gelu
    tables, dummy activations prefetch the switches).
  - PE term (Ph+bp)(Ph+bp)^T/16 carried rank-4 (u = p4 (Wp4 Wp4^T)/16 on
    the q side, p4 on the k side): exact, 36 of 64 score rows nonzero.
  - Power-governor aware: the ham duty-cycle throttle (k-of-8, ~3.4us
    windows) clamps sustained hot streams, so den/av (M=4/M=32) matmuls of
    batch b+1 space the fp8 bursts in batch b's gelu window and FFN2; the
    accumulated power debt is paid as a half-rate clamp over the final
    ~50us, so the tail also gets f16 outputs (host upcasts to f32) and a
    half-token drain to minimize clamped-region work.
"""

import functools

import numpy as np
import ml_dtypes

import concourse.bass as bass
import concourse.mybir as mybir
import concourse.tile as tile
from concourse import bacc
from concourse import hw_specs as _hw_specs
from concourse.bass_utils import run_bass_kernel_spmd

_KEEP_TABLES = ("natural_log_exp_and_others", "gelu_and_others")
_orig_get_tables = _hw_specs.get_activation_tables


@functools.cache
def _patched_get_tables(arch):
    tabs = _orig_get_tables(arch)
    return {k: (v if k in _KEEP_TABLES else set()) for k, v in tabs.items()}


_hw_specs.get_activation_tables = _patched_get_tables
bacc.get_activation_tables = _patched_get_tables

B, S, D, H, DH, DFF = 32, 512, 256, 8, 32, 2048
NCORES = 8
BL = B // NCORES
P = 128
DT = D // P     # 2 feature tiles
FT = DFF // P   # 16 ffn tiles
ST = S // P     # 4 seq tiles
f32 = mybir.dt.float32
f32r = mybir.dt.float32r
bf16 = mybir.dt.bfloat16
f16 = mybir.dt.float16
f8 = mybir.dt.float8e4
AF = mybir.ActivationFunctionType
ALU = mybir.AluOpType
DR = mybir.MatmulPerfMode.DoubleRow
EPS = 1e-5
W8SC = 16.0  # fp8 weight pre-scale


def build_nc(beta_zero):
    nc = bacc.Bacc("TRN2", target_bir_lowering=False, debug=False,
                   num_devices=NCORES)

    QT = nc.dram_tensor("QT", (BL, P, DT, S), f16, kind="ExternalInput")
    KT = nc.dram_tensor("KT", (BL, P, DT, S), f16, kind="ExternalInput")
    pT = nc.dram_tensor("pT", (BL, 4, S), f16, kind="ExternalInput")
    Wq = nc.dram_tensor("Wq", (P, DT, D), f16, kind="ExternalInput")
    Wv = nc.dram_tensor("Wv", (P, DT, D), f16, kind="ExternalInput")
    WqA = nc.dram_tensor("WqA", (P, DT, 4, P), f16, kind="ExternalInput")
    WkA = nc.dram_tensor("WkA", (P, DT, 4, P), f16, kind="ExternalInput")
    WpAq = nc.dram_tensor("WpAq", (4, 4, P), f16, kind="ExternalInput")
    WpAk = nc.dram_tensor("WpAk", (4, 4, P), f16, kind="ExternalInput")
    W1 = nc.dram_tensor("W1", (P, DT, DFF), f8, kind="ExternalInput")
    W2b = nc.dram_tensor("W2b", (P, FT, D), f8, kind="ExternalInput")
    bq = nc.dram_tensor("bq", (P, DT), f32, kind="ExternalInput")
    bvb = nc.dram_tensor("bvb", (P, D), f32, kind="ExternalInput")
    b1 = nc.dram_tensor("b1", (P, FT), f32, kind="ExternalInput")
    b2 = nc.dram_tensor("b2", (P, DT), f32, kind="ExternalInput")
    g0r = nc.dram_tensor("g0r", (1, D), f16, kind="ExternalInput")
    nb0 = nc.dram_tensor("nb0", (1, D), f16, kind="ExternalInput")
    g1r = nc.dram_tensor("g1r", (1, D), f16, kind="ExternalInput")
    nb1 = nc.dram_tensor("nb1", (1, D), f16, kind="ExternalInput")
    one33 = nc.dram_tensor("one33", (P, 2, 33), f16, kind="ExternalInput")
    Ed4 = nc.dram_tensor("Ed4", (P, 4, 4), bf16, kind="ExternalInput")
    EB4 = nc.dram_tensor("EB4", (4, P), bf16, kind="ExternalInput")
    onesS = nc.dram_tensor("onesS", (1, S), f16, kind="ExternalInput")
    outT = nc.dram_tensor("outT", (BL, P, DT, S), f16, kind="ExternalOutput")

    with tile.TileContext(nc) as tc:
        with (
            tc.tile_pool(name="singles", bufs=1) as singles,
            tc.tile_pool(name="dbl", bufs=2) as dbl,
            tc.tile_pool(name="ps_mm", bufs=2, space="PSUM") as ps_mm,
            tc.tile_pool(name="ps_sc", bufs=2, space="PSUM") as ps_sc,
            tc.tile_pool(name="ps_acc", bufs=1, space="PSUM") as ps_acc,
            tc.tile_pool(name="ps_av", bufs=1, space="PSUM") as ps_av,
        ):
            def load(dram, shape, eng=None):
                t = singles.tile(list(shape), dram.dtype, name="w_" + dram.name)
                (eng or nc.sync).dma_start(
                    t, dram[tuple(slice(None) for _ in shape)])
                return t

            # order matters: only what batch 0's proj needs loads first;
            # the FFN weights stream in during batch-0 attention
            Wq_sb = load(Wq, (P, DT, D))

            def loadj(dram, shape):
                st = load(dram, shape)
                t = singles.tile(list(shape), f32, name="j_" + dram.name)
                nc.vector.tensor_copy(t, st)
                return t

            bq_sb = loadj(bq, (P, DT))

            eps1 = singles.tile([1, 1], f32)
            nc.vector.memset(eps1, EPS)
            neghalf = singles.tile([1, 1], f32)
            nc.vector.memset(neghalf, -0.5)
            dummy = singles.tile([1, 1], f32)
            nc.vector.memset(dummy, 1.0)

            def ln_stats(x_sb):
                """Per-token mean/var stats chain -> (rstd, cst) tiles."""
                x2 = dbl.tile([P, DT, S], f16, tag="x2", bufs=1, name="x2")
                for t in range(DT):
                    nc.gpsimd.tensor_tensor(x2[:, t, :], x_sb[:, t, :],
                                            x_sb[:, t, :], ALU.mult)
                # partition 0 <- mean, partition 32 <- E[x^2]
                acc = ps_acc.tile([33, S], f32, tag="acc", name="acc")
                for t in range(DT):
                    nc.tensor.matmul(acc, one33_sb[:, 0, :], x_sb[:, t, :],
                                     start=(t == 0), stop=False)
                for t in range(DT):
                    nc.tensor.matmul(acc, one33_sb[:, 1, :], x2[:, t, :],
                                     start=False, stop=(t == DT - 1))
                rstd = dbl.tile([1, S], f16, tag="rstd", name="rstd")
                m2v = dbl.tile([1, S], f32r, tag="m2v", name="m2v")
                cst = dbl.tile([1, S], f16, tag="cst", name="cst")
                nc.scalar.activation(m2v, acc[0:1, :], AF.Square)
                nc.vector.tensor_sub(m2v, acc[32:33, :], m2v)
                nc.scalar.activation(acc[32:33, :], m2v, AF.Ln, bias=eps1)
                # rstd = exp(-0.5*ln(var+eps))
                nc.scalar.activation(rstd, acc[32:33, :], AF.Exp,
                                     scale=neghalf)
                # C = mean * rstd
                nc.vector.tensor_mul(cst, acc[0:1, :], rstd)
                return rstd, cst

            def layer_norm(x_sb, grow, nbrow, out_sb, filler=None,
                           split=False, sink=None, pre=None):
                """out = LN(x) * g + beta.  x_sb [P,DT,S] f16."""
                rstd, cst = pre if pre is not None else ln_stats(x_sb)
                layer_norm.rstd = rstd
                # independent matmuls emitted here keep the PE fed while the
                # Square->sub->Ln->Exp stats chain resolves (in-order queue)
                if filler is not None:
                    filler()
                # split=True emits the broadcast+tail per half-token chunk so
                # the final output DMAs start earlier (last-batch drain)
                halves = ((slice(0, S // 2), slice(S // 2, S))
                          if split else (slice(0, S),))
                for t in range(DT):
                    for sl in halves:
                        bcAC = ps_sc.tile([P, 2, S], f32, tag="sc",
                                          name="bcAC")
                        bcA, bcC = bcAC[:, 0, sl], bcAC[:, 1, sl]
                        nc.tensor.matmul(bcA, grow[0:1, t * P:(t + 1) * P],
                                         rstd[:, sl], start=True, stop=True)
                        nc.tensor.matmul(bcC, grow[0:1, t * P:(t + 1) * P],
                                         cst[:, sl], start=True,
                                         stop=beta_zero)
                        if not beta_zero:
                            nc.tensor.matmul(
                                bcC, nbrow[0:1, t * P:(t + 1) * P],
                                onesS_sb[:, sl], start=False, stop=True)
                        # out = x*(g*rstd) - (g*mean*rstd - beta)
                        nc.vector.tensor_mul(out_sb[:, t, sl],
                                             x_sb[:, t, sl], bcA)
                        nc.vector.tensor_sub(out_sb[:, t, sl],
                                             out_sb[:, t, sl], bcC)
                        if sink is not None:
                            sink(t, sl)

            def stage_load(b, stt):
                QT_sb = dbl.tile([P, DT, S], f16, tag="qt", name="QT_sb")
                nc.sync.dma_start(QT_sb, QT[b])
                KT_sb = dbl.tile([P, DT, S], f16, tag="kt", name="KT_sb")
                nc.sync.dma_start(KT_sb, KT[b])
                pT_sb = dbl.tile([4, S], f16, tag="pt", name="pT_sb")
                nc.sync.dma_start(pT_sb, pT[b])
                stt.update(QT=QT_sb, KT=KT_sb, pT=pT_sb)

            def stage_proj(b, stt):
                QT_sb, KT_sb, pT_sb = stt["QT"], stt["KT"], stt["pT"]
                # natural Qh (for the attention residual)
                Qh = dbl.tile([P, DT, S], f16, tag="qh", name="Qh")
                for t in range(DT):
                    ps = ps_mm.tile([P, S], f32, tag="mm", name="psq")
                    for kt in range(DT):
                        nc.tensor.matmul(
                            ps, Wq_sb[:, kt, t * P:(t + 1) * P],
                            QT_sb[:, kt, :],
                            start=(kt == 0), stop=(kt == DT - 1))
                    nc.vector.tensor_tensor(
                        Qh[:, t, :], ps,
                        bq_sb[:, t:t + 1].to_broadcast((P, S)), ALU.add)
                # aug tiles for scores: tile j partitions =
                # [Qh(2j) | Ph(2j) | Qh(2j+1) | Ph(2j+1)], biases folded via
                # the pT ones-row, so one K=64 matmul per (head, kt) yields
                # QK^T + PP^T in a single accumulation
                QA = dbl.tile([P, 4, S], f16, tag="qa", name="QA")
                KA = dbl.tile([P, 4, S], f16, tag="ka", name="KA")
                for j in range(4):
                    ps = ps_mm.tile([P, S], f32, tag="mm", name="psqa")
                    for kt in range(DT):
                        nc.tensor.matmul(ps, WqA_sb[:, kt, j, :],
                                         QT_sb[:, kt, :],
                                         start=(kt == 0), stop=False)
                    nc.tensor.matmul(ps, WpAq_sb[:, j, :], pT_sb,
                                     start=False, stop=True)
                    nc.vector.tensor_copy(QA[:, j, :], ps)
                    ps = ps_mm.tile([P, S], f32, tag="mm", name="pska")
                    for kt in range(DT):
                        nc.tensor.matmul(ps, WkA_sb[:, kt, j, :],
                                         KT_sb[:, kt, :],
                                         start=(kt == 0), stop=False)
                    nc.tensor.matmul(ps, WpAk_sb[:, j, :], pT_sb,
                                     start=False, stop=True)
                    nc.vector.tensor_copy(KA[:, j, :], ps)

                # V in natural layout [keys, feat], bf16, bias fused in move
                Vh = dbl.tile([P, ST, D], bf16, tag="vh", name="Vh")
                for st in range(ST):
                    ps = ps_mm.tile([P, S], f32, tag="mm", name="psv")
                    for kt in range(DT):
                        nc.tensor.matmul(
                            ps[:, :D], KT_sb[:, kt, st * P:(st + 1) * P],
                            Wv_sb[:, kt, :],
                            start=(kt == 0), stop=(kt == DT - 1))
                    nc.vector.tensor_add(Vh[:, st, :], ps[:, :D], bvb_sb)
                stt.update(Qh=Qh, QA=QA, KA=KA, Vh=Vh)

            def attn_quad_scores(b, stt, quad, ktp):
                """Scores + exp for one (quad, key-tile-pair)."""
                QA, KA = stt["QA"], stt["KA"]
                expS = stt["expS"][quad]
                for h4 in range(4):
                    base = 64 * (h4 % 2)
                    j = 2 * quad + h4 // 2
                    ps2 = ps_sc.tile([P, 2, S], f32, tag="sc", name="pssc")
                    for k2 in range(2):
                        kt = 2 * ktp + k2
                        nc.tensor.matmul(
                            ps2[:, k2, :],
                            KA[base:base + 64, j, kt * P:(kt + 1) * P],
                            QA[base:base + 64, j, :],
                            start=True, stop=True,
                            tile_position=(base, 0))
                    nc.scalar.activation(
                        expS[h4][:, 2 * ktp:2 * ktp + 2, :],
                        ps2, AF.Exp)

            def attn_quad_denav(b, stt, quad, ktp):
                Vh = stt["Vh"]
                expS = stt["expS"][quad]
                den, av = stt["den"][quad], stt["av"][quad]
                for h4 in range(4):
                    h = 4 * quad + h4
                    for k2 in range(2):
                        kt = 2 * ktp + k2
                        nc.tensor.matmul(
                            den, Ed4_sb[:, h4, :],
                            expS[h4][:, kt, :],
                            start=(kt == 0 and h4 == 0),
                            stop=(kt == ST - 1 and h4 == 3),
                            skip_group_check=True)
                        nc.tensor.matmul(
                            av[32 * h4:32 * h4 + 32, :],
                            Vh[:, kt, 32 * h:32 * h + 32],
                            expS[h4][:, kt, :],
                            start=(kt == 0), stop=(kt == ST - 1),
                            tile_position=(0, 32 * h4),
                            skip_group_check=True)

            def attn_quad_out(b, stt, quad):
                """1/den broadcast, divide, +Qh residual -> OT[:, quad, :]."""
                Qh, OT = stt["Qh"], stt["OT"]
                den, av = stt["den"][quad], stt["av"][quad]
                r4f = dbl.tile([4, S], f32, tag="r4f", name="r4f")
                nc.vector.reciprocal_approx_fast(r4f, den[0:4, :])
                r4 = dbl.tile([4, S], bf16, tag="r4", name="r4")
                nc.vector.tensor_copy(r4, r4f)
                bc2 = ps_sc.tile([P, 2, S], f32, tag="sc", name="bc2")
                bc = bc2[:, 0, :]
                nc.tensor.matmul(bc, EB4_sb, r4, start=True, stop=True)
                bcS = dbl.tile([P, S], f32, tag="bcs", name="bcS")
                nc.vector.tensor_copy(bcS, bc)
                nc.vector.tensor_mul(OT[:, quad, :], av, bcS)
                nc.vector.tensor_add(OT[:, quad, :], OT[:, quad, :],
                                     Qh[:, quad, :])

            def attn_alloc(stt):
                stt["OT"] = dbl.tile([P, DT, S], f16, tag="ot", name="OT")
                stt["expS"] = {}
                stt["den"] = {}
                stt["av"] = {}

            def attn_quad_alloc(stt, quad):
                stt["expS"][quad] = [
                    dbl.tile([P, ST, S], bf16, tag=f"e{i}", name=f"expS{i}")
                    for i in range(4)]
                stt["den"][quad] = ps_acc.tile([4, S], f32, tag="acc",
                                               name="den")
                stt["av"][quad] = ps_av.tile([P, S], f32, tag="av", name="av")

            def attn_scores_exps(b, stt, quad):
                """Scores + exps for a quad — MUST run in an ln_exp-table
                window (LN1/LN2 stats fillers), never inside the gelu loop,
                or the act-table chooser inserts 1.3us switches."""
                attn_quad_alloc(stt, quad)
                for ktp in range(ST // 2):
                    attn_quad_scores(b, stt, quad, ktp)

            def attn_denav(b, stt, quad):
                """den/av matmuls — PE-only (no ACT), safe anywhere."""
                for ktp in range(ST // 2):
                    attn_quad_denav(b, stt, quad, ktp)

            def stage_ffn(b, stt, nxt):
                OT = stt["OT"]
                # finish quad 1 of this batch's attention (scores+exps ran
                # in LN2(b-1)'s window) so OT is complete before LN1 stats
                attn_denav(b, stt, 1)
                attn_quad_out(b, stt, 1)

                LN1 = dbl.tile([P, DT, S], f16, tag="ln1", name="LN1")

                def filler():
                    if nxt is not None:
                        stage_proj(b + 1, nxt)
                        attn_alloc(nxt)
                        attn_scores_exps(b + 1, nxt, 0)
                layer_norm(OT, g0_sb, nb0_sb, LN1, filler=filler)
                # fp8 copy of LN1 for the DoubleRow FFN1 matmuls; per-t so
                # the t=0 copy overlaps the t=1 LN tail on the DVE queue
                LN1q = dbl.tile([P, DT, S], f8, tag="ln1q", name="LN1q")
                for t in range(DT):
                    nc.vector.tensor_copy(LN1q[:, t, :], LN1[:, t, :])
                # prefetch the gelu table; input dep on LN1's rstd pins this
                # after LN1's Exp in the ACT queue (scheduler can't hoist it)
                nc.scalar.activation(dummy, layer_norm.rstd[0:1, 0:1],
                                     AF.Gelu)

                # G loop (ACT-paced ~570ns/gelu vs 256ns of PE work per ft):
                # interleave batch b+1's den/av (PE-only) to keep the PE fed
                G = dbl.tile([P, FT, S], f8, tag="g", bufs=1, name="G")

                def g_chunk(ft0, ft1):
                    for ft in range(ft0, ft1):
                        ps = ps_mm.tile([P, S], f32, tag="mm", name="psf")
                        nc.tensor.matmul(
                            ps, W1_sb[:, :, ft * P:(ft + 1) * P],
                            LN1q, start=True, stop=True, perf_mode=DR)
                        nc.scalar.activation(G[:, ft, :], ps, AF.Gelu,
                                             bias=b1_sb[:, ft:ft + 1],
                                             scale=1.0 / W8SC)

                if nxt is not None:
                    # denav ktp0 mixes into the ACT-paced gelu window; ktp1
                    # mixes into the FFN2 stretch (spreads the hot fp8
                    # streams between power-light M=32/M=4 matmuls so the
                    # duty-cycle governor stays at full rate)
                    g_chunk(0, 8)
                    attn_quad_denav(b + 1, nxt, 0, 0)
                    g_chunk(8, FT)
                else:
                    g_chunk(0, FT)
                # prefetch the ln/exp table; dep on the last gelu's output
                # pins it after the gelu loop in the ACT queue
                nc.scalar.activation(dummy, G[0:1, FT - 1, 0:1], AF.Ln)
                Z = dbl.tile([P, DT, S], f16, tag="z", bufs=1, name="Z")
                for t in range(DT):
                    ps = ps_mm.tile([P, S], f32, tag="mm", name="psf2")
                    for fp in range(FT // 2):
                        nc.tensor.matmul(
                            ps, W2_sb[:, 2 * fp:2 * fp + 2,
                                      t * P:(t + 1) * P],
                            G[:, 2 * fp:2 * fp + 2, :],
                            start=(fp == 0), stop=(fp == FT // 2 - 1),
                            perf_mode=DR)
                    if t == 0 and nxt is not None:
                        attn_quad_denav(b + 1, nxt, 0, 1)
                    # Z = ps/16 + LN1 + b2
                    nc.vector.scalar_tensor_tensor(
                        Z[:, t, :], ps, 1.0 / W8SC, LN1[:, t, :],
                        ALU.mult, ALU.add)
                    nc.vector.tensor_tensor(
                        Z[:, t, :], Z[:, t, :],
                        b2_sb[:, t:t + 1].to_broadcast((P, S)), ALU.add)
                if nxt is not None:
                    attn_quad_out(b + 1, nxt, 0)
                stt["Z"] = Z

            def stage_out(b, stt, nxt):
                OUT = dbl.tile([P, DT, S], f16, tag="out", name="OUT")
                filler = None
                if nxt is not None:
                    def filler():
                        # input DMA for b+2 enqueued before this batch's
                        # output DMA (in-order sync queue head-of-line fix)
                        if b + 2 < BL:
                            stage_load(b + 2, sts[b + 2])
                        attn_scores_exps(b + 1, nxt, 1)
                    layer_norm(stt["Z"], g1_sb, nb1_sb, OUT, filler=filler)
                    for t in range(DT):
                        nc.sync.dma_start(outT[b][:, t, :], OUT[:, t, :])
                else:
                    # last batch: half-token tail so the output DMAs start
                    # as soon as each chunk's normalize completes
                    def sink(t, sl):
                        nc.sync.dma_start(outT[b][:, t, sl], OUT[:, t, sl])
                    layer_norm(stt["Z"], g1_sb, nb1_sb, OUT,
                               split=True, sink=sink)

            # software pipeline (steady state, per batch b):
            #   [denav+out q1(b)] [LN1 stats | proj(b+1) + scores/exps
            #   q0(b+1)] [LN1 bc] [gelu window # denav+out q0(b+1)]
            #   [FFN2] [LN2 stats | load(b+2) + scores/exps q1(b+1)]
            #   [LN2 bc] [outT]
            # so exps only ever run inside ln_exp-table windows and the
            # ACT-paced gelu stream overlaps PE-only den/av work.
            sts = [dict() for _ in range(BL)]
            stage_load(0, sts[0])
            # big weights go on the gpsimd SWDGE queue so they don't
            # serialize with the input/output DMAs on the sync queue
            WqA_sb = load(WqA, (P, DT, 4, P), eng=nc.gpsimd)
            WkA_sb = load(WkA, (P, DT, 4, P), eng=nc.gpsimd)
            WpAq_sb = load(WpAq, (4, 4, P))
            WpAk_sb = load(WpAk, (4, 4, P))
            Wv_sb = load(Wv, (P, DT, D), eng=nc.gpsimd)
            bvb_sb = loadj(bvb, (P, D))
            Ed4_sb = load(Ed4, (P, 4, 4))
            EB4_sb = load(EB4, (4, P))
            one33_sb = load(one33, (P, 2, 33))
            onesS_sb = load(onesS, (1, S))
            g0_sb = load(g0r, (1, D))
            nb0_sb = load(nb0, (1, D))
            g1_sb = load(g1r, (1, D))
            nb1_sb = load(nb1, (1, D))
            if BL > 1:
                stage_load(1, sts[1])
            stage_proj(0, sts[0])
            attn_alloc(sts[0])
            attn_scores_exps(0, sts[0], 0)
            attn_denav(0, sts[0], 0)
            attn_quad_out(0, sts[0], 0)
            attn_scores_exps(0, sts[0], 1)
            W1_sb = load(W1, (P, DT, DFF), eng=nc.gpsimd)
            W2_sb = load(W2b, (P, FT, D), eng=nc.gpsimd)
            b1_sb = loadj(b1, (P, FT))
            b2_sb = loadj(b2, (P, DT))
            for b in range(BL):
                nxt = sts[b + 1] if b + 1 < BL else None
                stage_ffn(b, sts[b], nxt)
                stage_out(b, sts[b], nxt)

    nc.finalize()
    return nc


_NC = None


def kernel(Q, K, p, Wq, bq, Wk, bk, Wv, bv, Wp, bp, g0, beta0, W1, b1, W2, b2,
           g1, beta1):
    global _NC
    beta_zero = bool(np.all(np.asarray(beta0) == 0)
                     and np.all(np.asarray(beta1) == 0))
    if _NC is None:
        _NC = build_nc(beta_zero)

    f = np.float32
    h16 = np.float16
    bf = ml_dtypes.bfloat16
    q8 = ml_dtypes.float8_e4m3

    def to8(x):
        return np.clip(np.asarray(x, f), -240.0, 240.0).astype(q8)

    def feat_tiles(x):  # [B, S, D] -> [B, P, DT, S]
        x = np.asarray(x, f).transpose(0, 2, 1).reshape(-1, DT, P, S)
        return np.ascontiguousarray(x.transpose(0, 2, 1, 3))

    def pp(vec, n):  # [n*P] -> [P, n]
        return np.ascontiguousarray(np.asarray(vec, f).reshape(n, P).T)

    def wmat(w, n, m):  # [n*P, m] -> [P, n, m]
        w = np.asarray(w, f).reshape(n, P, m)
        return np.ascontiguousarray(w.transpose(1, 0, 2))

    QTf = feat_tiles(Q)
    KTf = feat_tiles(K)
    # p padded to 4 channels; row 3 = ones (carries the PE-proj bias).
    # PE projection pre-scaled by 1/4 so PhPh^T carries the 1/sqrt(DV)=1/16.
    pTf = np.zeros((B, 4, S), f)
    pTf[:, :3, :] = np.transpose(np.asarray(p, f), (0, 2, 1))
    pTf[:, 3, :] = 1.0
    Wq_f = np.asarray(Wq, f)
    Wk_f = np.asarray(Wk, f)
    Wp_f = np.asarray(Wp, f)
    bq_f = np.asarray(bq, f)
    bk_f = np.asarray(bk, f)
    bp_f = np.asarray(bp, f)

    def aug_w(W):  # [D, D] -> [P, DT, 4, P] lhsT tiles
        out = np.zeros((P, DT, 4, P), f)
        Wt = W.reshape(DT, P, D)  # [kt, row, out_feature]
        for j in range(4):
            for hh in range(2):
                h = 2 * j + hh
                out[:, :, j, 64 * hh:64 * hh + 32] = \
                    Wt[:, :, 32 * h:32 * h + 32].transpose(1, 0, 2)
        return out

    def aug_p(bias, qside):
        """[4, 4, P] pT-projection lhsT.  The PE term (Ph+bp)(Ph+bp)^T/16 is
        rank 4 in p4=[p|1], so the aug slots carry 4 values instead of 32:
        q side gets u = p4 @ M4 (M4 = Wp4 Wp4^T / 16), k side gets p4
        verbatim; slots 36-63 stay zero (zero rows cost no PE power).
        Row 3 (pT's ones row) also broadcasts bq/bk into the Qh/Kh slots."""
        out = np.zeros((4, 4, P), f)
        for j in range(4):
            for hh in range(2):
                h = 2 * j + hh
                Wp4 = np.concatenate(
                    [Wp_f[:, 32 * h:32 * h + 32],
                     bp_f[None, 32 * h:32 * h + 32]], axis=0)  # [4, 32]
                if qside:
                    out[:, j, 64 * hh + 32:64 * hh + 36] = \
                        (Wp4 @ Wp4.T) / 16.0
                else:
                    for c in range(4):
                        out[c, j, 64 * hh + 32 + c] = 1.0
                out[3, j, 64 * hh:64 * hh + 32] += bias[32 * h:32 * h + 32]
        return out

    # EB4: r4 row h4 -> out partitions 32*h4..32*h4+31
    EB4m = np.zeros((4, P), f)
    for h4 in range(4):
        EB4m[h4, 32 * h4:32 * h4 + 32] = 1.0
    # Ed4[:, h4, :]: all-ones col h4 (masked partition-sum lhsT)
    Ed4m = np.zeros((P, 4, 4), f)
    for h4 in range(4):
        Ed4m[:, h4, h4] = 1.0
    # LN partition-sum weights (1/D folded in): [:,0,:] puts sum(x)/D at
    # out partition 0, [:,1,:] puts sum(x^2)/D at out partition 32
    one33m = np.zeros((P, 2, 33), f)
    one33m[:, 0, 0] = 1.0 / D
    one33m[:, 1, 32] = 1.0 / D

    shared = {
        "Wq": wmat(Wq, DT, D).astype(h16),
        "Wv": wmat(Wv, DT, D).astype(h16),
        "WqA": aug_w(Wq_f).astype(h16), "WkA": aug_w(Wk_f).astype(h16),
        "WpAq": aug_p(bq_f, True).astype(h16),
        "WpAk": aug_p(bk_f, False).astype(h16),
        "W1": to8(wmat(W1, DT, DFF) * W8SC),
        "W2b": to8(wmat(W2, FT, D) * W8SC),
        "bq": pp(bq, DT),
        "bvb": np.ascontiguousarray(np.broadcast_to(np.asarray(bv, f), (P, D))),
        "b1": pp(b1, FT), "b2": pp(b2, DT),
        "g0r": np.asarray(g0, f).reshape(1, D).astype(h16),
        "nb0": -np.asarray(beta0, f).reshape(1, D).astype(h16),
        "g1r": np.asarray(g1, f).reshape(1, D).astype(h16),
        "nb1": -np.asarray(beta1, f).reshape(1, D).astype(h16),
        "one33": one33m.astype(h16),
        "Ed4": Ed4m.astype(bf), "EB4": EB4m.astype(bf),
        "onesS": np.ones((1, S), h16),
    }
    in_maps = []
    for c in range(NCORES):
        m = dict(shared)
        m["QT"] = np.ascontiguousarray(QTf[c * BL:(c + 1) * BL]).astype(h16)
        m["KT"] = np.ascontiguousarray(KTf[c * BL:(c + 1) * BL]).astype(h16)
        m["pT"] = np.ascontiguousarray(pTf[c * BL:(c + 1) * BL]).astype(h16)
        in_maps.append(m)

    import os
    trace = bool(os.environ.get("BASS_TRACE"))
    res = run_bass_kernel_spmd(_NC, in_maps, core_ids=list(range(NCORES)),
                               trace=trace)
    kernel._LAST = res
    outs = [np.asarray(res.results[c]["outT"], np.float32)
            for c in range(NCORES)]
    full = np.concatenate(outs, axis=0)  # [B, P, DT, S]
    full = full.transpose(0, 2, 1, 3).reshape(B, D, S)  # [B, D, S]
    return np.ascontiguousarray(full.transpose(0, 2, 1))
